# revision 27
# baseline (speedup 1.0000x reference)
"""Trainium2 Bass kernel for the 2-layer GAT node-classification head.

The reference reads only h2[mask_idx] and x[mask_idx], so the exact
computation collapses to mask_idx's 2-hop in-neighborhood.  Math identity
used throughout: the layer-1 GAT aggregate per head

    agg_h[v] = sum_e alpha_e^h * (x[src_e] @ W1_h)
             = (sum_e alpha_e^h * x[src_e]) @ W1_h          (linearity)

so the big GEMM only needs v1p output rows per head, with W1 as the
*stationary* (lhsT) matmul operand — N = v1p per matmul, which makes the
tensor-engine time negligible; the kernel is DMA/latency bound (the per-core
W1 head block is the single big transfer, 1.2MB bf16).

Two-launch sharding over 8 cores (no collective — the runtime's
collective_compute costs a flat ~15us which dominated the previous version):

  launch A (8 cores): head h on core h.  Per-core: attention logits from
    attention-folded weights, segment softmax, alpha-weighted x combination,
    h1_agg = xcomb @ W1_h, elu, folded layer-2 partial h2f [v1p, 4] plus the
    x[mask] classifier partial, one [v1p, 6] output.
  host: concatenates the 8 partials (pure gather - no arithmetic).
  launch B (8 cores, redundant): sums the 8 head partials (one-hot matmul),
    layer-2 segment softmax at the mask node, final weighted sum.

DMA plan for A (DMA engines are serial; W1 must start early but after the
small attention operands): cb (wide 128-part, small) then w1 on the SP HWDGE
queue; the narrow few-partition operands (x rows, one-hot expansions) go
through the Pool SWDGE queue in parallel.

Host preprocessing: index-select/transposes of needed x rows (sharding),
attention/classifier folds into weight matrices (weights-only), one-hot
selection matrices for the segment softmax."""

import numpy as np
import ml_dtypes

import concourse.bass as bass
import concourse.mybir as mybir
import concourse.tile as tile
from concourse import bacc
from concourse.bass_utils import run_bass_kernel_spmd

NCORES = 8
P = 128
C = 768          # input feature dim
H1 = 8           # layer-1 heads
OUT = 768        # per-head feature dim
KC = C // P      # 6 contraction chunks of 128

f32 = mybir.dt.float32
bf16 = mybir.dt.bfloat16
BF = ml_dtypes.bfloat16


def _pad(n, q):
    return max(q, ((n + q - 1) // q) * q)


# ---------------------------------------------------------------- host graph
def _preprocess(edge_index, mask_idx, n_nodes):
    """2-hop in-neighborhood of mask_idx; build one-hot matrices so segment
    softmax lowers to matmuls.  Everything in meta is compile-time python."""
    ei = np.asarray(edge_index).astype(np.int64)
    m = int(np.asarray(mask_idx))
    src_all = np.concatenate([ei[0], np.arange(n_nodes, dtype=np.int64)])
    dst_all = np.concatenate([ei[1], np.arange(n_nodes, dtype=np.int64)])

    s1_pos = np.nonzero(dst_all == m)[0]          # in-edges of m (incl loop)
    s1_src = src_all[s1_pos].tolist()
    v1 = list(dict.fromkeys(s1_src))              # unique sources
    v1n = len(v1)
    v1p = v1n           # no padding: a padded group would have a zero
    # softmax denominator (1/0 -> inf -> 0*inf NaN in the edge expansion)
    assert v1n <= P, f"mask in-degree (unique) too large: {v1n}"
    assert NCORES * v1p <= P, f"v1p {v1p} too large for launch-B stacking"
    v1_row = {v: r for r, v in enumerate(v1)}
    s1n = len(s1_src)
    n_s1t = max(1, -(-s1n // P))
    s1p = n_s1t * P
    assert s1p <= 512, f"mask in-degree {s1n} exceeds 512"

    # S2: in-edges of each v in V1, uniform stride gmax within 128-slot tiles
    groups = [src_all[np.nonzero(dst_all == v)[0]].tolist() for v in v1]
    gmax = max(len(g) for g in groups)
    assert gmax <= P, f"in-degree {gmax} exceeds {P}"
    gpt = P // gmax
    n_et = -(-v1n // gpt)
    s2p = n_et * P

    # unique node set U (v1 first so dst rows exist)
    u_list = list(v1)
    u_row = dict(v1_row)
    for g in groups:
        for s in g:
            if s not in u_row:
                u_row[s] = len(u_list)
                u_list.append(s)
    un = len(u_list)
    n_ut = -(-un // P)
    assert n_ut <= 2, f"unique 2-hop sources {un} exceed 256"
    ut_sizes = [min(P, _pad(un - t * P, 4)) if t == n_ut - 1 else P
                for t in range(n_ut)]

    s_src = np.zeros((un, s2p), np.float32)       # one-hot src expansion
    s_dst = np.zeros((un, s2p), np.float32)       # one-hot dst expansion
    m01 = np.zeros((s2p, v1p), np.float32)        # slot -> group one-hot
    for t in range(n_et):
        gs = groups[t * gpt:(t + 1) * gpt]
        for j, srcs in enumerate(gs):
            vr = t * gpt + j
            lo = t * P + j * gmax
            for k, s in enumerate(srcs):
                s_src[u_row[s], lo + k] = 1.0
                s_dst[u_row[v1[vr]], lo + k] = 1.0
                m01[lo + k, vr] = 1.0

    # S1 side (for launch B)
    g_mat = np.zeros((v1p, s1p), np.float32)      # a_src2 gather by edge
    gm_mat = np.zeros((v1p, s1p), np.float32)     # a_dst2 broadcast (row m)
    for e, s in enumerate(s1_src):
        g_mat[v1_row[s], e] = 1.0
        gm_mat[v1_row[m], e] = 1.0

    meta = dict(m=m, v1n=v1n, v1p=v1p, s1n=s1n, s1p=s1p, n_s1t=n_s1t,
                n_et=n_et, gmax=gmax, un=un, n_ut=n_ut,
                ut_sizes=tuple(ut_sizes))
    host = dict(u_list=u_list, s_src=s_src, s_dst=s_dst, m01=m01,
                g=g_mat, gm=gm_mat)
    return meta, host


def _chunk6(w):
    """[768, n] -> [128, 6, n] contraction-chunked."""
    n = w.shape[1]
    return np.ascontiguousarray(w.reshape(KC, P, n).transpose(1, 0, 2))


def _layout(pieces):
    lay, off = {}, 0
    for name, rows, cols in pieces:
        lay[name] = (rows, off, cols)
        off += cols
    return lay, off


def _a_layouts(meta):
    v1p, n_et, n_ut = meta["v1p"], meta["n_et"], meta["n_ut"]
    s2p = n_et * P
    uts = meta["ut_sizes"]
    ucols = sum(uts)
    u_top = uts[0]
    bpieces = [                                    # cb: bf16, 128-part, small
        ("xcolT", P, KC * ucols),                  # x^T chunks per u-tile
        ("wswd", P, KC * 2),                       # [Ws1_h | Wd1_h] chunked
        ("xm", P, KC * v1p),                       # x[mask] chunked, replic.
        ("wfb", P, KC * 2),                        # fc_w bottom fold, chunked
        ("w2f4", P, KC * 4),                       # folded layer-2 rhs cols
        ("m01", P, n_et * v1p),                    # slot -> group one-hot
        ("ssrcT", P, n_et * ucols),                # per (et, ut) [128, u_pad]
    ]
    npieces = [                                    # nb: bf16, u_top-part
        ("xrow", u_top, n_ut * C),                 # x rows per u-tile
        ("ssrc", u_top, n_ut * s2p),
        ("sdst", u_top, n_ut * s2p),
        ("m01T", v1p, s2p),
        ("b1r", 1, OUT),                           # b1 head slice, row
        ("bias3", 1, 2),
        ("onesv", 1, v1p),
        ("nw2s", 1, 4),                            # -colsum(w2f4)
    ]
    blay, bw = _layout(bpieces)
    nlay, nw = _layout(npieces)
    return blay, bw, nlay, nw, u_top


def _b_layout(meta):
    v1p, s1p = meta["v1p"], meta["s1p"]
    rows = NCORES * v1p
    pieces = [                                     # f32, 8*v1p partitions
        ("stk", rows, 4),                          # concatenated A partials
        ("g", rows, s1p),                          # G tiled 8x (sums heads)
        ("gm", rows, s1p),
        ("g02", rows, s1p),                        # 0.2-scaled for leaky
        ("gm02", rows, s1p),
        ("oxmb", 2, 1),                            # x_m partial + folded bias
        ("ones2", rows, 2),                        # 1/8 (pm via g colsums)
    ]
    return _layout(pieces)


# ---------------------------------------------------------------- launch A
def _build_a(meta):
    v1p, n_et, n_ut = meta["v1p"], meta["n_et"], meta["n_ut"]
    s2p = n_et * P
    uts = meta["ut_sizes"]
    ucols = sum(uts)
    ustart = [sum(uts[:t]) for t in range(n_ut)]
    blay, bw, nlay, nw, u_top = _a_layouts(meta)

    nc = bacc.Bacc("TRN2", target_bir_lowering=False, debug=False,
                   enable_asserts=True, num_devices=NCORES)
    d_cb = nc.dram_tensor("cb", [P, bw], bf16, kind="ExternalInput")
    d_nb = nc.dram_tensor("nb", [u_top, nw], bf16, kind="ExternalInput")
    d_w1a = nc.dram_tensor("w1a", [P, 3 * OUT], bf16, kind="ExternalInput")
    d_w1b = nc.dram_tensor("w1b", [P, 3 * OUT], bf16, kind="ExternalInput")
    d_part = nc.dram_tensor("part", [v1p, 6], f32, kind="ExternalOutput")

    with tile.TileContext(nc) as tc:
        with (
            tc.tile_pool(name="const", bufs=1) as cpool,
            tc.tile_pool(name="sbuf", bufs=2) as sb,
            tc.tile_pool(name="ps", bufs=1, space="PSUM") as ps,
            nc.allow_low_precision(
                reason="bf16 rounding is within the 2e-2 rel-err budget"),
        ):
            cb = cpool.tile([P, bw], bf16, tag="cb")
            nc.sync.dma_start(out=cb[:], in_=d_cb[:])
            w1a_sb = cpool.tile([P, 3 * OUT], bf16, tag="w1a")
            nc.sync.dma_start(out=w1a_sb[:], in_=d_w1a[:])
            w1b_sb = cpool.tile([P, 3 * OUT], bf16, tag="w1b")
            nc.sync.dma_start(out=w1b_sb[:], in_=d_w1b[:])
            nb = cpool.tile([u_top, nw], bf16, tag="nb")
            nc.gpsimd.dma_start(out=nb[:], in_=d_nb[:])

            def bv(name):
                rows, off, cols = blay[name]
                return cb[0:rows, off:off + cols]

            def nv(name):
                rows, off, cols = nlay[name]
                return nb[0:rows, off:off + cols]

            xcolT_v = bv("xcolT").rearrange("p (k n) -> p k n", k=KC)
            wswd_v = bv("wswd").rearrange("p (k n) -> p k n", k=KC)
            xm_v = bv("xm").rearrange("p (k n) -> p k n", k=KC)
            wfb_v = bv("wfb").rearrange("p (k n) -> p k n", k=KC)
            w2f4_v = bv("w2f4").rearrange("p (k n) -> p k n", k=KC)
            m01_v = bv("m01").rearrange("p (t n) -> p t n", t=n_et)
            ssrcT_v = bv("ssrcT")
            xrow_v = nv("xrow")
            ssrc_v = nv("ssrc")
            sdst_v = nv("sdst")
            m01T_v = nv("m01T")
            b1r_v = nv("b1r")
            bias3_v = nv("bias3")
            onesv_v = nv("onesv")
            nw2s_v = nv("nw2s")

            # ---- a_su = x_u @ [Ws1_h | Wd1_h] per u-tile ----
            asu_sb = []
            for t in range(n_ut):
                up = uts[t]
                asp = ps.tile([up, 2], f32, tag=f"asu{t}", name=f"asu{t}")
                for c in range(KC):
                    nc.tensor.matmul(
                        out=asp[:],
                        lhsT=xcolT_v[:, c, ustart[t]:ustart[t] + up],
                        rhs=wswd_v[:, c, :],
                        start=(c == 0), stop=(c == KC - 1))
                asb = sb.tile([up, 2], bf16, tag=f"asu_sb{t}",
                              name=f"asu_sb{t}")
                nc.vector.tensor_copy(out=asb[:], in_=asp[:])
                asu_sb.append(asb)

            # ---- logits per edge slot, as columns [128, n_et] ----
            raw_ps = ps.tile([P, n_et], f32, tag="raw", name="raw")
            for t in range(n_et):
                for ti, ut in enumerate(range(n_ut)):
                    up = uts[ut]
                    nc.tensor.matmul(
                        out=raw_ps[:, t:t + 1],
                        lhsT=ssrc_v[0:up, ut * s2p + t * P:
                                    ut * s2p + (t + 1) * P],
                        rhs=asu_sb[ut][:, 0:1],
                        start=(ti == 0), stop=False)
                    nc.tensor.matmul(
                        out=raw_ps[:, t:t + 1],
                        lhsT=sdst_v[0:up, ut * s2p + t * P:
                                    ut * s2p + (t + 1) * P],
                        rhs=asu_sb[ut][:, 1:2],
                        start=False, stop=(ti == n_ut - 1))
            # leaky relu (slope .2) then exp; padding slots are killed by m01
            lk = sb.tile([P, n_et], f32, tag="lk")
            nc.vector.tensor_scalar_mul(out=lk[:], in0=raw_ps[:], scalar1=0.2)
            nc.vector.tensor_tensor(out=lk[:], in0=raw_ps[:], in1=lk[:],
                                    op=mybir.AluOpType.max)
            expc = sb.tile([P, n_et], bf16, tag="expc")
            nc.scalar.activation(out=expc[:], in_=lk[:],
                                 func=mybir.ActivationFunctionType.Exp)

            # ---- denominators per group, reciprocal, edge expansion ----
            den_ps = ps.tile([v1p, 1], f32, tag="den", name="den")
            for t in range(n_et):
                nc.tensor.matmul(out=den_ps[:], lhsT=m01_v[:, t, :],
                                 rhs=expc[:, t:t + 1],
                                 start=(t == 0), stop=(t == n_et - 1))
            recip = sb.tile([v1p, 1], bf16, tag="recip")
            nc.vector.reciprocal(out=recip[:], in_=den_ps[:])
            # expand 1/denom back to edge slots (alpha must be normalized
            # BEFORE the elu nonlinearity downstream)
            edge_ps = ps.tile([P, n_et], f32, tag="edge", name="edge")
            for t in range(n_et):
                nc.tensor.matmul(out=edge_ps[:, t:t + 1],
                                 lhsT=m01T_v[:, t * P:(t + 1) * P],
                                 rhs=recip[:], start=True, stop=True)
            acol = sb.tile([P, n_et], f32, tag="acol")
            nc.vector.tensor_tensor(out=acol[:], in0=expc[:], in1=edge_ps[:],
                                    op=mybir.AluOpType.mult)

            # ---- A0[u, v] = sum_e S_src[u,e] alpha_e m01[e,v] ----
            em01 = sb.tile([P, n_et, v1p], bf16, tag="em01")
            for t in range(n_et):
                nc.vector.tensor_scalar(out=em01[:, t, :], in0=m01_v[:, t, :],
                                        scalar1=acol[:, t:t + 1], scalar2=None,
                                        op0=mybir.AluOpType.mult)
            a_bf = []
            for ut in range(n_ut):
                up = uts[ut]
                a0 = ps.tile([up, v1p], f32, tag=f"asu{ut}", name=f"a0_{ut}")
                for t in range(n_et):
                    nc.tensor.matmul(
                        out=a0[:],
                        lhsT=ssrcT_v[:, t * ucols + ustart[ut]:
                                     t * ucols + ustart[ut] + up],
                        rhs=em01[:, t, :],
                        start=(t == 0), stop=(t == n_et - 1))
                ab = sb.tile([up, v1p], bf16, tag=f"a_bf{ut}",
                             name=f"a_bf{ut}")
                nc.vector.tensor_copy(out=ab[:], in_=a0[:])
                a_bf.append(ab)

            # ---- xcombT[c, v] = sum_u x[u, c] A0[u, v] ----
            xc_ps = ps.tile([P, KC * v1p], f32, tag="xc", name="xc")
            for c in range(KC):
                for ut in range(n_ut):
                    up = uts[ut]
                    nc.tensor.matmul(
                        out=xc_ps[:, c * v1p:(c + 1) * v1p],
                        lhsT=xrow_v[0:up, ut * C + c * P:
                                    ut * C + (c + 1) * P],
                        rhs=a_bf[ut][:],
                        start=(ut == 0), stop=(ut == n_ut - 1))
            xcT = sb.tile([P, KC, v1p], bf16, tag="xcT")
            nc.vector.tensor_copy(
                out=xcT[:].rearrange("p k n -> p (k n)"), in_=xc_ps[:])

            # ---- x_m classifier partial (independent of W1/chain) ----
            out_ps = ps.tile([v1p, 6], f32, tag="out", name="out_ps")
            for c in range(KC):
                nc.tensor.matmul(out=out_ps[:, 4:6], lhsT=xm_v[:, c, :],
                                 rhs=wfb_v[:, c, :],
                                 start=(c == 0), stop=False)
            nc.tensor.matmul(out=out_ps[:, 4:6],
                             lhsT=onesv_v, rhs=bias3_v,
                             start=False, stop=True)

            # ---- GEMM1 + b1: h1T[f, v] = W1^T xcomb + b1, W1 stationary.
            # Two half-passes so the c=0..2 matmuls overlap the second W1
            # DMA; each half is a closed accumulation group per region.
            h1_ps = ps.tile([P, KC * v1p], f32, tag="h1", name="h1")
            for fc in range(KC):
                for c in range(KC):
                    w1h = w1a_sb if c < 3 else w1b_sb
                    nc.tensor.matmul(
                        out=h1_ps[:, fc * v1p:(fc + 1) * v1p],
                        lhsT=w1h[:, (c % 3) * OUT + fc * P:
                                 (c % 3) * OUT + (fc + 1) * P],
                        rhs=xcT[:, c, :],
                        start=(c == 0), stop=False)
                nc.tensor.matmul(
                    out=h1_ps[:, fc * v1p:(fc + 1) * v1p],
                    lhsT=b1r_v[:, fc * P:(fc + 1) * P],
                    rhs=onesv_v, start=False, stop=True)

            # ---- elu(z) = exp(min(z,0)) + max(z,0) - 1; the -1 folds into
            # a host-side -colsum(w2f4) constant in the layer-2 contraction --
            mn = sb.tile([P, KC * v1p], f32, tag="mn")
            nc.vector.tensor_scalar_min(out=mn[:], in0=h1_ps[:], scalar1=0.0)
            mx = sb.tile([P, KC, v1p], bf16, tag="mx")
            nc.vector.tensor_scalar_max(
                out=mx[:].rearrange("p k n -> p (k n)"), in0=h1_ps[:],
                scalar1=0.0)
            ez = sb.tile([P, KC, v1p], bf16, tag="ez")
            nc.scalar.activation(out=ez[:].rearrange("p k n -> p (k n)"),
                                 in_=mn[:],
                                 func=mybir.ActivationFunctionType.Exp)

            # ---- folded layer-2 partial (elu components + const fold) ----
            for c in range(KC):
                nc.tensor.matmul(out=out_ps[:, 0:4], lhsT=mx[:, c, :],
                                 rhs=w2f4_v[:, c, :],
                                 start=(c == 0), stop=False)
            for c in range(KC):
                nc.tensor.matmul(out=out_ps[:, 0:4], lhsT=ez[:, c, :],
                                 rhs=w2f4_v[:, c, :],
                                 start=False, stop=False)
            nc.tensor.matmul(out=out_ps[:, 0:4], lhsT=onesv_v,
                             rhs=nw2s_v, start=False, stop=True)
            stg = sb.tile([v1p, 6], f32, tag="stg")
            nc.vector.tensor_copy(out=stg[:], in_=out_ps[:])
            nc.sync.dma_start(out=d_part[:], in_=stg[:])

    nc.compile()
    return nc


# ---------------------------------------------------------------- launch B
def _build_b(meta):
    v1p, s1p, n_s1t = meta["v1p"], meta["s1p"], meta["n_s1t"]
    lay, cw = _b_layout(meta)
    rows = NCORES * v1p

    nc = bacc.Bacc("TRN2", target_bir_lowering=False, debug=False,
                   enable_asserts=True, num_devices=NCORES)
    d_cst = nc.dram_tensor("cst", [rows, cw], f32, kind="ExternalInput")
    d_res = nc.dram_tensor("res", [2, 1], f32, kind="ExternalOutput")

    with tile.TileContext(nc) as tc:
        with (
            tc.tile_pool(name="const", bufs=1) as cpool,
            tc.tile_pool(name="sbuf", bufs=2) as sb,
            tc.tile_pool(name="ps", bufs=1, space="PSUM") as ps,
        ):
            cst = cpool.tile([rows, cw], f32, tag="cst")
            nc.sync.dma_start(out=cst[:], in_=d_cst[:])

            def cv(name):
                r, off, cols = lay[name]
                return cst[0:r, off:off + cols]

            stk_v = cv("stk")
            g_v = cv("g")
            gm_v = cv("gm")
            g02_v = cv("g02")
            gm02_v = cv("gm02")
            oxmb_v = cv("oxmb")
            ones2_v = cv("ones2")

            # padmask columns (x2) = g column sums (1 on real S1 edges)
            pm_ps = ps.tile([P, n_s1t, 2], f32, tag="pm", name="pm")
            for t in range(n_s1t):
                nc.tensor.matmul(out=pm_ps[:, t, :],
                                 lhsT=g_v[:, t * P:(t + 1) * P],
                                 rhs=ones2_v, start=True, stop=True)
            pm = sb.tile([P, n_s1t, 2], f32, tag="pm_sb")
            nc.vector.tensor_copy(
                out=pm[:].rearrange("p a b -> p (a b)"),
                in_=pm_ps[:].rearrange("p a b -> p (a b)"))

            # raw logits (+0.2-scaled copy, interleaved so leaky relu is one
            # reduce_max) and the classifier gather, all reading the stacked
            # partials directly — Gbig absorbs the head summation
            raw_ps = ps.tile([P, n_s1t, 2], f32, tag="raw", name="raw")
            g2_ps = ps.tile([P, n_s1t * 2], f32, tag="g2", name="g2")
            for t in range(n_s1t):
                nc.tensor.matmul(out=raw_ps[:, t, 0:1],
                                 lhsT=g_v[:, t * P:(t + 1) * P],
                                 rhs=stk_v[:, 2:3], start=True, stop=False)
                nc.tensor.matmul(out=raw_ps[:, t, 0:1],
                                 lhsT=gm_v[:, t * P:(t + 1) * P],
                                 rhs=stk_v[:, 3:4], start=False, stop=True)
                nc.tensor.matmul(out=raw_ps[:, t, 1:2],
                                 lhsT=g02_v[:, t * P:(t + 1) * P],
                                 rhs=stk_v[:, 2:3], start=True, stop=False)
                nc.tensor.matmul(out=raw_ps[:, t, 1:2],
                                 lhsT=gm02_v[:, t * P:(t + 1) * P],
                                 rhs=stk_v[:, 3:4], start=False, stop=True)
                nc.tensor.matmul(out=g2_ps[:, 2 * t:2 * t + 2],
                                 lhsT=g_v[:, t * P:(t + 1) * P],
                                 rhs=stk_v[:, 0:2], start=True, stop=True)
            g2 = sb.tile([P, n_s1t * 2], f32, tag="g2sb")
            nc.vector.tensor_copy(out=g2[:], in_=g2_ps[:])

            lk = sb.tile([P, n_s1t], f32, tag="lk")
            nc.vector.reduce_max(out=lk[:], in_=raw_ps[:],
                                 axis=mybir.AxisListType.X)
            ex = sb.tile([P, n_s1t], f32, tag="ex")
            nc.scalar.activation(out=ex[:], in_=lk[:],
                                 func=mybir.ActivationFunctionType.Exp)

            # column orientation: den and res as [2, 1] so the final
            # normalize+bias is one fused tensor_scalar
            den_ps = ps.tile([2, 1], f32, tag="den", name="den")
            res_ps = ps.tile([2, 1], f32, tag="res", name="res")
            for t in range(n_s1t):
                nc.tensor.matmul(out=den_ps[:], lhsT=pm[:, t, :],
                                 rhs=ex[:, t:t + 1], start=(t == 0),
                                 stop=(t == n_s1t - 1))
                nc.tensor.matmul(out=res_ps[:],
                                 lhsT=g2[:, 2 * t:2 * t + 2],
                                 rhs=ex[:, t:t + 1], start=(t == 0),
                                 stop=(t == n_s1t - 1))
            rc = sb.tile([2, 1], f32, tag="rc")
            nc.vector.reciprocal(out=rc[:], in_=den_ps[:])
            res_sb = sb.tile([2, 1], f32, tag="res_sb")
            nc.vector.tensor_scalar(out=res_sb[:], in0=res_ps[:],
                                    scalar1=rc[:], scalar2=oxmb_v,
                                    op0=mybir.AluOpType.mult,
                                    op1=mybir.AluOpType.add)
            nc.sync.dma_start(out=d_res[:], in_=res_sb[:])

    nc.compile()
    return nc


_CACHE = {}


def _get_ncs(meta):
    key = repr(sorted(meta.items()))
    if key not in _CACHE:
        _CACHE[key] = (_build_a(meta), _build_b(meta))
    return _CACHE[key]


# ---------------------------------------------------------------- host prep
def make_a_inputs(meta, host, inputs):
    x = np.asarray(inputs["x"], np.float32)
    W1 = np.asarray(inputs["W1"], np.float32)
    att_s1 = np.asarray(inputs["att_src1"], np.float32)
    att_d1 = np.asarray(inputs["att_dst1"], np.float32)
    b1 = np.asarray(inputs["b1"], np.float32)
    W2 = np.asarray(inputs["W2"], np.float32)
    att_s2 = np.asarray(inputs["att_src2"], np.float32)
    att_d2 = np.asarray(inputs["att_dst2"], np.float32)
    b2 = np.asarray(inputs["b2"], np.float32)
    fc_w = np.asarray(inputs["fc_w"], np.float32)
    fc_b = np.asarray(inputs["fc_b"], np.float32)
    cls_w = np.asarray(inputs["cls_w"], np.float32)
    cls_b = np.asarray(inputs["cls_b"], np.float32)

    Ws1 = np.einsum("chf,hf->ch", W1.reshape(C, H1, OUT), att_s1)  # [C, H1]
    Wd1 = np.einsum("chf,hf->ch", W1.reshape(C, H1, OUT), att_d1)
    Ws2 = W2 @ att_s2[0]                                           # [H1*OUT]
    Wd2 = W2 @ att_d2[0]
    wf = fc_w @ cls_w                                              # [1536, 2]
    wf_top, wf_bot = wf[:OUT], wf[OUT:]
    w2fold = W2 @ wf_top                                           # [6144, 2]
    bias3 = (b2 @ wf_top + fc_b @ cls_w + cls_b).reshape(1, 2)

    v1p, n_et, n_ut = meta["v1p"], meta["n_et"], meta["n_ut"]
    s2p = n_et * P
    uts = meta["ut_sizes"]
    ucols = sum(uts)
    ustart = [sum(uts[:t]) for t in range(n_ut)]
    un = meta["un"]
    blay, bw, nlay, nw, u_top = _a_layouts(meta)

    u_ids = np.array(host["u_list"], np.int64)
    xu = x[u_ids]                                                  # [un, 768]

    xcolT = np.zeros((P, KC, ucols), np.float32)
    xrow = np.zeros((u_top, n_ut * C), np.float32)
    ssrc = np.zeros((u_top, n_ut * s2p), np.float32)
    sdst = np.zeros((u_top, n_ut * s2p), np.float32)
    ssrcT = np.zeros((P, n_et * ucols), np.float32)
    for t in range(n_ut):
        lo, hi = t * P, min(un, (t + 1) * P)
        w = hi - lo
        blk = xu[lo:hi]                                            # [w, 768]
        xcolT[:, :, ustart[t]:ustart[t] + w] = (
            blk.T.reshape(KC, P, w).transpose(1, 0, 2))
        xrow[0:w, t * C:(t + 1) * C] = blk
        ssrc[0:w, t * s2p:(t + 1) * s2p] = host["s_src"][lo:hi]
        sdst[0:w, t * s2p:(t + 1) * s2p] = host["s_dst"][lo:hi]
        for et in range(n_et):
            ssrcT[:, et * ucols + ustart[t]:et * ucols + ustart[t] + w] = (
                host["s_src"][lo:hi, et * P:(et + 1) * P].T)

    xm1 = x[meta["m"]].reshape(KC, P).T                            # [128, 6]
    xm = np.repeat(xm1.reshape(P, KC, 1), v1p, axis=2).reshape(P, KC * v1p)

    def fill(cstm, lay, name, arr):
        rows, off, cols = lay[name]
        assert arr.shape == (rows, cols), (name, arr.shape, (rows, cols))
        cstm[0:rows, off:off + cols] = arr

    in_maps = []
    for i in range(NCORES):
        h = i % H1
        w1blk = W1[:, h * OUT:(h + 1) * OUT]                       # [768, 768]
        wswd = np.stack([Ws1[:, h], Wd1[:, h]], axis=1)            # [768, 2]
        w2f4 = np.concatenate(
            [w2fold[h * OUT:(h + 1) * OUT],
             Ws2[h * OUT:(h + 1) * OUT, None],
             Wd2[h * OUT:(h + 1) * OUT, None]], axis=1)            # [768, 4]
        cbm = np.zeros((P, bw), np.float32)
        fill(cbm, blay, "xcolT", xcolT.reshape(P, KC * ucols))
        fill(cbm, blay, "wswd", _chunk6(wswd).reshape(P, KC * 2))
        fill(cbm, blay, "xm", xm)
        fill(cbm, blay, "wfb", _chunk6(wf_bot).reshape(P, KC * 2))
        fill(cbm, blay, "w2f4", _chunk6(w2f4).reshape(P, KC * 4))
        fill(cbm, blay, "m01",
             np.concatenate([host["m01"][t * P:(t + 1) * P]
                             for t in range(n_et)], axis=1))
        fill(cbm, blay, "ssrcT", ssrcT)
        nbm = np.zeros((u_top, nw), np.float32)
        fill(nbm, nlay, "xrow", xrow)
        fill(nbm, nlay, "ssrc", ssrc)
        fill(nbm, nlay, "sdst", sdst)
        fill(nbm, nlay, "m01T", np.ascontiguousarray(host["m01"].T))
        fill(nbm, nlay, "b1r", b1[h * OUT:(h + 1) * OUT].reshape(1, OUT))
        fill(nbm, nlay, "bias3", bias3.astype(np.float32))
        fill(nbm, nlay, "onesv", np.ones((1, v1p), np.float32))
        fill(nbm, nlay, "nw2s", -w2f4.sum(axis=0).reshape(1, 4))
        # w1 in stationary-chunk layout: [128, c*768 + f], split in halves
        w1l = w1blk.reshape(KC, P, OUT).transpose(1, 0, 2)
        w1a = np.ascontiguousarray(w1l[:, 0:3].reshape(P, 3 * OUT))
        w1b = np.ascontiguousarray(w1l[:, 3:6].reshape(P, 3 * OUT))
        im = {"cb": cbm.astype(BF), "nb": nbm.astype(BF),
              "w1a": w1a.astype(BF), "w1b": w1b.astype(BF)}
        in_maps.append(im)
    return in_maps


def make_b_inputs(meta, host, parts):
    """parts: list of 8 per-core [v1p, 6] arrays from launch A."""
    v1p = meta["v1p"]
    lay, cw = _b_layout(meta)
    rows = NCORES * v1p
    cstm = np.zeros((rows, cw), np.float32)
    s1p = meta["s1p"]

    def fill(name, arr):
        r, off, cols = lay[name]
        assert arr.shape == (r, cols), (name, arr.shape, (r, cols))
        cstm[0:r, off:off + cols] = arr

    stk = np.concatenate([np.asarray(p, np.float32)[0:v1p, 0:4]
                          for p in parts], axis=0)                 # [8v1p, 4]
    gbig = np.tile(host["g"], (NCORES, 1))
    gmbig = np.tile(host["gm"], (NCORES, 1))
    fill("stk", stk)
    fill("g", gbig)
    fill("gm", gmbig)
    fill("g02", 0.2 * gbig)
    fill("gm02", 0.2 * gmbig)
    fill("oxmb", np.ascontiguousarray(
        np.asarray(parts[0], np.float32)[0:1, 4:6].T))
    fill("ones2", np.full((rows, 2), 1.0 / NCORES, np.float32))
    return [{"cst": cstm} for _ in range(NCORES)]


# ---------------------------------------------------------------- entrypoint
def kernel(**inputs):
    x = np.asarray(inputs["x"], np.float32)
    meta, host = _preprocess(inputs["edge_index"], inputs["mask_idx"],
                             x.shape[0])
    nc_a, nc_b = _get_ncs(meta)
    in_a = make_a_inputs(meta, host, inputs)
    res_a = run_bass_kernel_spmd(nc_a, in_a, core_ids=list(range(NCORES)))
    parts = [res_a.results[i]["part"] for i in range(NCORES)]
    in_b = make_b_inputs(meta, host, parts)
    res_b = run_bass_kernel_spmd(nc_b, in_b, core_ids=list(range(NCORES)))
    return np.ascontiguousarray(
        res_b.results[0]["res"].reshape(2)[None, :]).astype(np.float32)
